# revision 32
# baseline (speedup 1.0000x reference)
"""DAN classifier (embedding gather + mean-pool + tiny MLP + batch log-softmax)
on 8 Trainium2 NeuronCores.

Sharding: data-parallel over the batch (sentence) dim — 2048 sentences/core.
The embedding table is quantized to fp8 e4m3 (x1024 scale, folded back out
in the MLP weights) and padded to [400000, 512] so rows are 512 B — the DMA
small-transfer threshold. The tiny MLP weights stay f32 and are replicated.

Per-core device kernel (8 sets of 2 groups x 128 sentences, 12800 tok/set):
  - The vocab is split into 13 buckets of 32768 rows so row indices fit the
    int16 index format of InstDMAGatherAnt. One dma_gather per (set, bucket)
    (104 ops vs 208 at group granularity — the ~1us/op descriptor-generation
    fixed cost on GPSIMD is the reason to batch). Budgets are per
    (set, bucket, group) maxima across cores so the SPMD block structure is
    compile-time static; each core pads its slot lists with index-0 dummies
    (emb row 0 is all zeros).
  - Slot layout per (set, bucket): group 0's tokens (padded to bud0), then
    group 1's (padded to bud1). Gathered slot k -> partition k%128, block
    k//128 of a [128, nblk, 512] fp8 tile.
  - Pooling: a per-set one-hot S[k, blk, s] = (sent_local[k, blk] == s),
    built on DVE in fp8. Per 128-slot block one fp8 matmul S_blk.T @ G_blk
    accumulates into PSUM f32 [128 sentences, 300]; the block containing the
    static g0/g1 boundary runs two partial-K matmuls into the two groups'
    PSUM tiles. fp8 keeps the PE at 1 cycle/row (fp32 is 4).
  - MLP (f32): PE transpose of pooled -> [300, 128]; matmuls against
    V_w.T/(SEQ*SCALE) (mean + fp8 scale fold), ReLU+bias on ACT, W matmul,
    W_b add on DVE.
  - One DMA writes logits.T [2, 2048] to DRAM.

Host glue: shard/pack tokens, run SPMD on cores 0-7, concatenate the logit
slabs and apply the global log-softmax over the batch axis (16384x2 —
negligible next to the on-device gather).
"""

import numpy as np

VOCAB, DIM, HID, OUT = 400000, 300, 32, 2
BATCH, SEQ = 16384, 50
N_CORES = 8
B_CORE = BATCH // N_CORES            # 2048 sentences per core
GROUP = 128                          # sentences per group
N_GROUPS = B_CORE // GROUP           # 16
G_SET = 2                            # groups per gather set
N_SETS = N_GROUPS // G_SET           # 8
EPAD = 512                           # fp8 row bytes (512B DMA threshold)
SCALE = 1024.0                       # emb pre-scale before fp8 quantization
BUCKET = 32768                       # int16-addressable rows per bucket
NB = -(-VOCAB // BUCKET)             # 13
DCH = (128, 128, DIM - 256)          # contraction chunks over DIM
N_QUEUES = 4


def _cdiv(a, b):
    return -(-a // b)


class _Plan:
    """Per-(set,bucket,group) budgets and packed-layout offsets shared by
    the host packer and the device builder."""

    def __init__(self, budgets):
        self.budgets = budgets            # [n_sets][NB][G_SET] ints
        self.icol_off = []                # idx col offset per (s,b)
        self.blk_off = []                 # slot blk offset per (s,b)
        self.nblk_s = []                  # blocks per set
        io = 0
        bo = 0
        for s in range(len(budgets)):
            row_i, row_b = [], []
            blk0 = bo
            for b in range(NB):
                n = sum(budgets[s][b])
                row_i.append(io)
                row_b.append(bo)
                io += _cdiv(n, 16)
                bo += _cdiv(n, 128)
            self.icol_off.append(row_i)
            self.blk_off.append(row_b)
            self.nblk_s.append(bo - blk0)
        self.icols_tot = io
        self.nblk_tot = bo
        self.max_nblk_s = max(self.nblk_s)
        self.max_nblk_b = max(_cdiv(sum(bs), 128) for row in budgets
                              for bs in row)

    def key(self):
        return tuple(tuple(tuple(g) for g in r) for r in self.budgets)


def _build_bass(plan, vocab=VOCAB, dim=DIM, hid=HID, nout=OUT,
                b_core=B_CORE, group=GROUP, n_cores=N_CORES):
    from contextlib import ExitStack

    import concourse.tile as tile
    from concourse import bacc, mybir

    f32 = mybir.dt.float32
    f8 = mybir.dt.float8e4
    i16 = mybir.dt.int16
    dch = DCH
    nch = len(dch)

    nc = bacc.Bacc("TRN2", target_bir_lowering=False, debug=False,
                   enable_asserts=False, num_devices=n_cores,
                   num_swdge_queues=N_QUEUES)
    t_idx = nc.declare_dram_parameter("gidx", [128, plan.icols_tot], i16,
                                      isOutput=False)
    t_sent = nc.declare_dram_parameter("sent", [128, plan.nblk_tot], f32,
                                       isOutput=False)
    t_iota = nc.declare_dram_parameter("iota", [128, group], f32,
                                       isOutput=False)
    t_ident = nc.declare_dram_parameter("ident", [128, 128], f32,
                                        isOutput=False)
    t_emb = nc.declare_dram_parameter("embp", [vocab, EPAD], f8,
                                      isOutput=False)
    t_vwt = nc.declare_dram_parameter("vwt", [128, nch * hid], f32,
                                      isOutput=False)
    t_vb = nc.declare_dram_parameter("vb", [hid, 1], f32, isOutput=False)
    t_wwt = nc.declare_dram_parameter("wwt", [hid, nout], f32, isOutput=False)
    t_wb = nc.declare_dram_parameter("wb", [nout, 1], f32, isOutput=False)
    t_out = nc.declare_dram_parameter("out", [nout, b_core], f32,
                                      isOutput=True)

    relu = mybir.ActivationFunctionType.Relu
    is_eq = mybir.AluOpType.is_equal

    with ExitStack() as ctx:
        tc = ctx.enter_context(tile.TileContext(nc))
        consts = ctx.enter_context(tc.tile_pool(name="consts", bufs=1))
        gpool = ctx.enter_context(tc.tile_pool(name="gather", bufs=2 * NB - 2))
        spool = ctx.enter_context(tc.tile_pool(name="smat", bufs=3))
        sbp = ctx.enter_context(tc.tile_pool(name="sbwork", bufs=2))
        # pooled_sb tiles live from set s until mlp(s) at iteration s+2:
        # 3 sets x 2 groups in flight.
        plp = ctx.enter_context(tc.tile_pool(name="pooledsb", bufs=6))
        pp_pool = ctx.enter_context(tc.tile_pool(name="ppool", bufs=3, space="PSUM"))
        pt_pool = ctx.enter_context(tc.tile_pool(name="ptpool", bufs=2, space="PSUM"))
        ph_pool = ctx.enter_context(tc.tile_pool(name="phpool", bufs=1, space="PSUM"))
        pl_pool = ctx.enter_context(tc.tile_pool(name="plpool", bufs=1, space="PSUM"))
        pd_pool = ctx.enter_context(tc.tile_pool(name="pdpool", bufs=1, space="PSUM"))

        # idx table loaded per set (separate tiles) so the first gathers
        # wait only on set 0's slice, not the whole 1.7MB load.
        idx_tiles = []
        for s in range(N_SETS):
            lo = plan.icol_off[s][0]
            hi = plan.icol_off[s + 1][0] if s + 1 < N_SETS else plan.icols_tot
            it = consts.tile([128, hi - lo], i16, name=f"idx{s}")
            nc.sync.dma_start(it[:], t_idx[:, lo:hi])
            idx_tiles.append((it, lo))
        sent_sb = consts.tile([128, plan.nblk_tot], f32)
        nc.sync.dma_start(sent_sb[:], t_sent[:])
        iota_sb = consts.tile([128, group], f32)
        nc.sync.dma_start(iota_sb[:], t_iota[:])
        ident = consts.tile([128, 128], f32)
        nc.sync.dma_start(ident[:], t_ident[:])
        vwt_sb = consts.tile([128, nch * hid], f32)
        nc.sync.dma_start(vwt_sb[:], t_vwt[:])
        vb_sb = consts.tile([hid, 1], f32)
        nc.sync.dma_start(vb_sb[:], t_vb[:])
        wwt_sb = consts.tile([hid, nout], f32)
        nc.sync.dma_start(wwt_sb[:], t_wwt[:])
        wb_sb = consts.tile([nout, 1], f32)
        nc.sync.dma_start(wb_sb[:], t_wb[:])
        out_sb = consts.tile([nout, b_core], f32)

        # Compute instructions carry at most ONE embedded sync wait after
        # codegen. Prime each engine's vector clock on every external
        # producer it will consume mid-loop, so steady-state instructions
        # need only the wait on their data tile.
        dumb_dve = consts.tile([hid, 1], f32)
        nc.vector.tensor_copy(dumb_dve[0:nout, :], wb_sb[:])
        nc.vector.tensor_copy(dumb_dve[:], sent_sb[0:hid, 0:1])
        nc.vector.tensor_copy(dumb_dve[:], iota_sb[0:hid, 0:1])
        dumb_act = consts.tile([hid, 1], f32)
        nc.scalar.copy(dumb_act[:], vb_sb[:])
        dumb_ps = pd_pool.tile([1, 1], f32)
        nc.tensor.matmul(dumb_ps[:], lhsT=ident[:, 0:1], rhs=ident[:, 0:1],
                         start=True, stop=True)
        nc.tensor.matmul(dumb_ps[:], lhsT=vwt_sb[:, 0:1], rhs=vwt_sb[:, 0:1],
                         start=True, stop=True)
        nc.tensor.matmul(dumb_ps[:], lhsT=wwt_sb[:, 0:1], rhs=wwt_sb[:, 0:1],
                         start=True, stop=True)

        def build_s(s):
            """One-hot S for all blocks of set s: S[k, blk, c] =
            (sent_local[k, blk] == c), one DVE op, fp8 out."""
            nblk = plan.nblk_s[s]
            s_t = spool.tile([128, plan.max_nblk_s * group], f8, tag="S")
            boff = plan.blk_off[s][0]
            in0 = sent_sb[:, boff:boff + nblk].to_broadcast([128, nblk, group])
            in1 = (iota_sb[:].rearrange("p (a c) -> p a c", a=1)
                   .to_broadcast([128, nblk, group]))
            nc.vector.tensor_tensor(
                out=s_t[:, 0:nblk * group].rearrange("p (c s) -> p c s",
                                                     s=group),
                in0=in0, in1=in1, op=is_eq)
            return s_t

        s_tiles = {0: build_s(0), 1: build_s(1)}
        # prime PE on the DVE-built S
        nc.tensor.matmul(dumb_ps[:], lhsT=s_tiles[0][:, 0:1],
                         rhs=s_tiles[0][:, 0:1], start=True, stop=True)

        def mlp(s, pooled_tiles):
            """MLP for set s (pooled already in SBUF). Called two sets late
            so none of its DVE/PE deps sit on the current set's critical
            path."""
            for j in range(G_SET):
                g = s * G_SET + j
                pooled_sb = pooled_tiles[j]
                pt_ps = pt_pool.tile([128, nch * group], f32, tag="pt")
                for c, w in enumerate(dch):
                    nc.tensor.transpose(
                        out=pt_ps[0:w, c * group: (c + 1) * group],
                        in_=pooled_sb[:, c * 128: c * 128 + w],
                        identity=ident[:group, :group],
                    )
                # pt/l glue runs on ACT, NOT DVE: anything MLP-related on the
                # in-order DVE ends up scheduled behind the ~16us is_eq and
                # transitively stalls the PE's pooling matmuls.
                pt_sb = sbp.tile([128, nch * group], f32, tag="pt_sb")
                nc.scalar.copy(pt_sb[:, 0:2 * group], pt_ps[:, 0:2 * group])
                nc.scalar.copy(pt_sb[0:dch[2], 2 * group:3 * group],
                               pt_ps[0:dch[2], 2 * group:3 * group])

                h_ps = ph_pool.tile([hid, group], f32, tag="h")
                for c, w in enumerate(dch):
                    nc.tensor.matmul(
                        h_ps[:],
                        lhsT=vwt_sb[0:w, c * hid: (c + 1) * hid],
                        rhs=pt_sb[0:w, c * group: (c + 1) * group],
                        start=(c == 0),
                        stop=(c == nch - 1),
                    )
                h_sb = sbp.tile([hid, group], f32, tag="h_sb")
                nc.scalar.activation(h_sb[:], h_ps[:], relu,
                                     bias=vb_sb[:, 0:1])

                l_ps = pl_pool.tile([nout, group], f32, tag="l")
                nc.tensor.matmul(l_ps[:], lhsT=wwt_sb[:], rhs=h_sb[:],
                                 start=True, stop=True)
                nc.scalar.add(out_sb[:, g * group: (g + 1) * group],
                              l_ps[:], wb_sb[:, 0:1])

        gather_ct = 0
        pooled_sbs = {}
        for s in range(N_SETS):
            # MLP for set s-2 first: its inputs are long since ready, so the
            # PE runs it without stalling before this set's matmuls, and its
            # DVE copies precede this set's is_eq.
            if s - 2 in pooled_sbs:
                mlp(s - 2, pooled_sbs.pop(s - 2))

            gtiles = []
            for b in range(NB):
                n = sum(plan.budgets[s][b])
                if n == 0:
                    gtiles.append(None)
                    continue
                gt = gpool.tile([128, plan.max_nblk_b * EPAD], f8, tag="G")
                rows = min(BUCKET, vocab - b * BUCKET)
                idx_sb, idx_lo = idx_tiles[s]
                io = plan.icol_off[s][b] - idx_lo
                # InstDMAGatherAnt dies above 1024 indices per op, and the
                # GPSIMD convoy pays full per-op overhead regardless of op
                # size — so split oversized (set,bucket) ranges into EQUAL
                # 128-aligned halves rather than 1024+remainder.
                if n <= 1024:
                    cuts = [(0, n)]
                else:
                    mid = (_cdiv(n, 128) // 2) * 128
                    cuts = [(0, mid), (mid, n - mid)]
                for c0, cn in cuts:
                    blk0 = c0 // 128
                    nblk = _cdiv(cn, 128)
                    nc.gpsimd.dma_gather(
                        out_ap=gt[:, blk0 * EPAD:(blk0 + nblk) * EPAD]
                        .rearrange("p (c e) -> p c e", e=EPAD),
                        in_ap=t_emb[b * BUCKET: b * BUCKET + rows, :],
                        idxs_ap=idx_sb[:, io + c0 // 16:
                                       io + c0 // 16 + _cdiv(cn, 16)],
                        num_idxs=cn,
                        num_idxs_reg=cn,
                        elem_size=EPAD,
                        queue_num=gather_ct % N_QUEUES,
                    )
                    gather_ct += 1
                gtiles.append(gt)

            s_s = s_tiles.pop(s)
            # S built TWO sets ahead: the list scheduler serializes the
            # ~16us is_eq behind this set's pooled copies on the in-order
            # DVE, so one set of slack is not enough — with two, the is_eq
            # finishing mid-set-s+1 still beats set s+2's matmuls.
            if s + 2 < N_SETS:
                s_tiles[s + 2] = build_s(s + 2)

            # per-(block, group-half) matmul list: (b, blk, r0, r1, j)
            mms = []
            for b in range(NB):
                bud0 = plan.budgets[s][b][0]
                bud1 = sum(plan.budgets[s][b][1:])
                n = bud0 + bud1
                if n == 0:
                    continue
                for blk in range(_cdiv(n, 128)):
                    lo, hi = blk * 128, min(blk * 128 + 128, n)
                    if lo < bud0:
                        mms.append((b, blk, 0, min(hi, bud0) - lo, 0))
                    if hi > bud0:
                        mms.append((b, blk, max(lo, bud0) - lo, hi - lo, 1))
            # Keep each PSUM accumulation group contiguous on the PE: all of
            # group 0's matmuls (one closed start..stop group), then group
            # 1's. Interleaving two open accumulation groups crashes the HW.
            mms.sort(key=lambda m: m[4])
            n_mm = [sum(1 for m in mms if m[4] == 0),
                    sum(1 for m in mms if m[4] == 1)]

            pooled_ps = [pp_pool.tile([group, dim], f32, tag="pooled",
                                      name=f"pooled{j}")
                         for j in range(G_SET)]
            mm_ct = [0, 0]
            for b, blk, r0, r1, j in mms:
                gt = gtiles[b]
                sblk = plan.blk_off[s][b] - plan.blk_off[s][0] + blk
                nc.tensor.matmul(
                    pooled_ps[j][:],
                    lhsT=s_s[r0:r1, sblk * group: (sblk + 1) * group],
                    rhs=gt[r0:r1, blk * EPAD: blk * EPAD + dim],
                    start=(mm_ct[j] == 0),
                    stop=(mm_ct[j] == n_mm[j] - 1),
                    skip_group_check=True,
                )
                mm_ct[j] += 1

            tiles = []
            for j in range(G_SET):
                pooled_sb = plp.tile([group, dim], f32, tag="pooled_sb",
                                     name=f"pooled_sb{j}")
                nc.vector.tensor_copy(pooled_sb[:], pooled_ps[j][:])
                tiles.append(pooled_sb)
            pooled_sbs[s] = tiles

        for s in sorted(pooled_sbs):
            mlp(s, pooled_sbs[s])

        nc.sync.dma_start(t_out[:], out_sb[:])
    nc.finalize()
    return nc


def _pack_weights(V_w, V_b, W_w, W_b, dim=DIM, hid=HID, nout=OUT, seq=SEQ):
    nch = len(DCH)
    vwt = (np.asarray(V_w, np.float32).T /
           np.float32(seq * SCALE)).astype(np.float32)
    vwt_packed = np.zeros((128, nch * hid), np.float32)
    off = 0
    for c, w in enumerate(DCH):
        vwt_packed[0:w, c * hid: (c + 1) * hid] = vwt[off: off + w]
        off += w
    wwt = np.ascontiguousarray(np.asarray(W_w, np.float32).T)
    vb = np.asarray(V_b, np.float32).reshape(hid, 1)
    wb = np.asarray(W_b, np.float32).reshape(nout, 1)
    return vwt_packed, vb, wwt, wb


def _plan_and_pack(tokens, b_core=B_CORE, group=GROUP, seq=SEQ):
    """Bucket every core's tokens at (set, bucket, group) granularity;
    compute cross-core budgets; pack int16 index and local-sentence-id
    tables per core."""
    n_cores = tokens.shape[0] // b_core
    toks = np.asarray(tokens, np.int64).reshape(
        n_cores, N_SETS, G_SET, group, seq)

    # per (core, set, group): flat token list + bucket of each token
    flat = toks.reshape(n_cores, N_SETS, G_SET, group * seq)
    sent_of = np.broadcast_to(np.arange(group)[:, None],
                              (group, seq)).reshape(group * seq)
    buck = flat >> 15
    counts = np.zeros((n_cores, N_SETS, NB, G_SET), np.int64)
    for b in range(NB):
        counts[:, :, b, :] = (buck == b).sum(axis=3).transpose(0, 1, 2)
    budgets = counts.max(axis=0)                     # [N_SETS, NB, G_SET]
    # Non-zero matmul base partitions crash the HW (found empirically, even
    # though bass allows 0/32/64): round the g0 budget up to full 128-blocks
    # so the static g0/g1 boundary sits on a block edge and every pooling
    # matmul starts at partition 0.
    budgets[:, :, 0] = -(-budgets[:, :, 0] // 128) * 128
    plan = _Plan(budgets.tolist())

    gidx = np.zeros((n_cores, 128, plan.icols_tot), np.int16)
    sent = np.full((n_cores, 128, plan.nblk_tot), -1.0, np.float32)
    for c in range(n_cores):
        for s in range(N_SETS):
            order = [np.argsort(buck[c, s, j], kind="stable")
                     for j in range(G_SET)]
            stoks = [flat[c, s, j][order[j]] for j in range(G_SET)]
            ssent = [sent_of[order[j]] for j in range(G_SET)]
            pos = [0] * G_SET
            for b in range(NB):
                buds = [int(budgets[s, b, j]) for j in range(G_SET)]
                tot = sum(buds)
                if tot == 0:
                    continue
                loc = np.zeros(tot, np.int16)
                sen = np.full(tot, -1.0, np.float32)
                off = 0
                for j in range(G_SET):
                    nj = int(counts[c, s, b, j])
                    loc[off:off + nj] = (stoks[j][pos[j]:pos[j] + nj]
                                         & 32767).astype(np.int16)
                    sen[off:off + nj] = ssent[j][pos[j]:pos[j] + nj]
                    pos[j] += nj
                    off += buds[j]
                # wrap idx: slot i -> [i % 16, io + i // 16]
                cols = _cdiv(tot, 16)
                w = np.zeros(cols * 16, np.int16)
                w[:tot] = loc
                io = plan.icol_off[s][b]
                gidx[c, :, io:io + cols] = np.tile(
                    w.reshape(cols, 16).T, (8, 1))
                # sent: slot k -> [k % 128, bo + k // 128]
                nblk = _cdiv(tot, 128)
                sw = np.full(nblk * 128, -1.0, np.float32)
                sw[:tot] = sen
                bo = plan.blk_off[s][b]
                sent[c, :, bo:bo + nblk] = sw.reshape(nblk, 128).T
    return plan, gidx, sent


_STATE = {}


def kernel(tokens, emb, V_w, V_b, W_w, W_b, _trace=False):
    import ml_dtypes
    from concourse.bass_utils import run_bass_kernel_spmd

    tokens = np.asarray(tokens)
    emb = np.asarray(emb, np.float32)

    plan, gidx, sent = _plan_and_pack(tokens)
    vwt_packed, vb, wwt, wb = _pack_weights(V_w, V_b, W_w, W_b)

    embp = _STATE.get("embp")
    if embp is None or _STATE.get("embp_src") is not emb:
        embp = np.zeros((VOCAB, EPAD), ml_dtypes.float8_e4m3)
        embp[:, :DIM] = (emb * np.float32(SCALE)).astype(
            ml_dtypes.float8_e4m3)
        embp[0, :] = 0.0  # padding_idx row stays exactly zero
        _STATE["embp"] = embp
        _STATE["embp_src"] = emb

    iota = np.broadcast_to(np.arange(GROUP, dtype=np.float32),
                           (128, GROUP)).copy()
    ident = np.eye(128, dtype=np.float32)

    nc = None
    if _STATE.get("plan_key") == plan.key():
        nc = _STATE.get("nc")
    if nc is None:
        nc = _build_bass(plan)
        _STATE["nc"] = nc
        _STATE["plan_key"] = plan.key()

    in_maps = [
        {
            "gidx": np.ascontiguousarray(gidx[c]),
            "sent": np.ascontiguousarray(sent[c]),
            "iota": iota,
            "ident": ident,
            "embp": embp,
            "vwt": vwt_packed,
            "vb": vb,
            "wwt": wwt,
            "wb": wb,
        }
        for c in range(N_CORES)
    ]
    res = run_bass_kernel_spmd(nc, in_maps, core_ids=list(range(N_CORES)),
                               trace=_trace)
    _STATE["last_result"] = res

    logits = np.concatenate([r["out"].T for r in res.results], axis=0)

    # global log-softmax over the batch axis (LogSoftmax(dim=0))
    x = logits.astype(np.float64)
    m = x.max(axis=0, keepdims=True)
    lse = m + np.log(np.sum(np.exp(x - m), axis=0, keepdims=True))
    return (x - lse).astype(np.float32)
